# revision 14
# baseline (speedup 1.0000x reference)
"""Trainium2 Bass kernel for nn_ClassicalEncoderDecoder.

The reference applies 8 sequential "rings" of 1024 Givens rotations each
(4 encoder + 4 decoder), with a sigmoid-gated blend in the middle:

    b          = x @ E                      (E = enc ring composite)
    bottleneck = (1-w) * b + w * hs
    out        = bottleneck @ D             (D = dec ring composite)

Everything is linear in x, so the whole computation collapses to two
matmuls with a rank-1 bias:

    bottleneck = x @ [(1-w) E]      + w * hs
    out        = x @ [(1-w) E D]    + w * (hs @ D)

The composite matrices are O(n^2) and derived from the tiny angle
parameters (O(n)); they are built on host in float64 by applying the
ring rotations to an identity matrix.  All O(B n^2) work (the matmuls
over the batch) runs on the 8 NeuronCores, data-parallel over the batch
dimension (no inter-core communication needed).
"""

import os
from contextlib import ExitStack

import numpy as np

import concourse.bass as bass
import concourse.mybir as mybir
import concourse.tile as tile
from concourse.bass_utils import run_bass_kernel_spmd
from concourse.vector_clock import ScopedClock

N_CORES = 8
B_FULL = 8192
NF = 1024            # feature dim
BS = B_FULL // N_CORES   # batch rows per core
NOUT = 2 * NF        # concat of [bottleneck | out] feature columns
F32 = mybir.dt.float32

# ---------------------------------------------------------------------------
# Tile/walrus workaround: this container's walrus rejects instructions that
# carry more than one semaphore wait ("Too many sync wait commands").  Tile's
# add_semaphores freely attaches several waits to one instruction, so after
# scheduling we split extra waits onto single-wait nops placed immediately
# before the instruction on the same engine.
# ---------------------------------------------------------------------------

_TILE_PSEUDO_CLASSES = tuple(
    c
    for c in (
        getattr(tile, "BassTileRelease", None),
        getattr(tile, "BassTileCriticalSection", None),
        getattr(tile, "TileBranchInst", None),
        getattr(tile, "BassTileLoopBlock", None),
        getattr(tile, "BassTileBranchHintPlaceholder", None),
    )
    if c is not None
)


def _split_excess_waits(nc, insts):
    out = []
    for inst in insts:
        si = getattr(inst, "sync_info", None)
        waits = list(si.on_wait) if si is not None else []
        eng = getattr(inst, "engine", None)
        if (
            len(waits) > 1
            and not isinstance(inst, _TILE_PSEUDO_CLASSES)
            and eng is not None
            and eng != mybir.EngineType.Unassigned
        ):
            for w in waits[:-1]:
                out.append(
                    mybir.InstNoOp(
                        name=nc.get_next_instruction_name(),
                        ins=[],
                        outs=[],
                        engine=eng,
                        sync_info=mybir.SyncInfo(on_wait=[w], on_update=[]),
                        bass_nofuse=True,
                    )
                )
            inst.sync_info = mybir.SyncInfo(
                on_wait=[waits[-1]], on_update=list(si.on_update)
            )
        out.append(inst)
    return out


_ORIG_LOWER_ORDERED = tile.TileContext._lower_ordered_insts


def _patched_lower_ordered_insts(self, ordered):
    for bb_name in list(ordered.keys()):
        ordered[bb_name] = _split_excess_waits(self.nc, ordered[bb_name])
    return _ORIG_LOWER_ORDERED(self, ordered)


if getattr(tile.TileContext._lower_ordered_insts, "__name__", "") != "_patched_lower_ordered_insts":
    tile.TileContext._lower_ordered_insts = _patched_lower_ordered_insts


def _patched_drain_and_barrier(self, tick_clock, wait_clock):
    nc = self.nc
    probe = nc.sync.nop(nofuse=True)
    wait_clock.add_sem_waits(probe.ins, ScopedClock({None: tick_clock.global_clock}))
    si = probe.ins.sync_info
    waits = list(si.on_wait) if si is not None else []
    if len(waits) > 1:
        probe.ins.sync_info = mybir.SyncInfo(on_wait=[waits[0]], on_update=[])
        for w in waits[1:]:
            n = nc.sync.nop(nofuse=True)
            n.ins.sync_info = mybir.SyncInfo(on_wait=[w], on_update=[])
    nc.sync.drain()
    nc.all_engine_barrier()
    popped = nc._tile_sem_poison_stack.pop()
    assert popped is self._sem_poison
    nc.clear_and_free_semaphores(list(self.sems.allocated().values()))
    nc.all_engine_barrier()


if getattr(tile.TileContext._drain_and_barrier, "__name__", "") != "_patched_drain_and_barrier":
    tile.TileContext._drain_and_barrier = _patched_drain_and_barrier


# ---------------------------------------------------------------------------
# Host-side composite-rotation precompute (float64, O(n^2))
# ---------------------------------------------------------------------------


def _ring_T_inplace(XT: np.ndarray, angles: np.ndarray) -> None:
    """Apply one ring of Givens rotations in the transposed domain.

    Mirrors reference._apply_ring's scan: XT rows are features; the result
    represents XT <- M^T @ XT where apply_ring(x) == x @ M.
    """
    n = angles.shape[0]
    c = np.cos(angles)
    s = np.sin(angles)
    for k in range(n - 1, -1, -1):
        j = k + 1 if k + 1 < n else 0
        xi = XT[k].copy()
        xj = XT[j]
        XT[k] = c[k] * xi - s[k] * xj
        XT[j] = s[k] * xi + c[k] * xj


def _host_params(angles_enc, angles_dec, hidden_weight, hidden_state):
    """Build W [NF, 2*NF] and bias [2*NF] (both float32)."""
    n = NF
    ET = np.eye(n, dtype=np.float64)
    for blk in range(angles_enc.shape[0]):
        _ring_T_inplace(ET, angles_enc[blk].astype(np.float64))
    # Decoder pass applied to [E^T | hs] gives [ (E D)^T | D^T hs ].
    A = np.concatenate([ET, hidden_state.astype(np.float64)[:, None]], axis=1)
    for blk in range(angles_dec.shape[0]):
        _ring_T_inplace(A, angles_dec[blk].astype(np.float64))
    EDT, dhs = A[:, :n], A[:, n]
    w = 1.0 / (1.0 + np.exp(-np.float64(hidden_weight[0])))
    W = np.empty((n, NOUT), np.float32)
    W[:, :n] = ((1.0 - w) * ET.T).astype(np.float32)
    W[:, n:] = ((1.0 - w) * EDT.T).astype(np.float32)
    bias = np.concatenate(
        [w * hidden_state.astype(np.float64), w * dhs]
    ).astype(np.float32)
    return W, bias


# ---------------------------------------------------------------------------
# Device program
# ---------------------------------------------------------------------------

_MM_MODE = os.environ.get("KERNEL_MM_DTYPE", "f32r")
_MM_DTYPE = {
    "f32": mybir.dt.float32,
    "f32r": mybir.dt.float32r,
    "bf16": mybir.dt.bfloat16,
}[_MM_MODE]
# numpy dtype for the xt / wm host arrays fed to the device
_IO_NP_DTYPE = mybir.dt.np(_MM_DTYPE)


def _build_program():
    nc = bass.Bass(trn_type="TRN2")
    xt = nc.dram_tensor("xt", [NF, BS], _MM_DTYPE, kind="ExternalInput")
    wm = nc.dram_tensor("wm", [NF, NOUT], _MM_DTYPE, kind="ExternalInput")
    bv = nc.dram_tensor("bv", [NOUT], F32, kind="ExternalInput")
    out = nc.dram_tensor("out", [BS, NOUT], F32, kind="ExternalOutput")

    KT = NF // 128   # 8 contraction tiles
    MT = BS // 128   # 8 batch row tiles
    NT = NOUT // 512  # 4 psum-bank-wide column tiles

    with tile.TileContext(nc) as tc, ExitStack() as ctx:
        const = ctx.enter_context(tc.tile_pool(name="const", bufs=1))
        psum = ctx.enter_context(tc.tile_pool(name="psum", bufs=2, space="PSUM"))
        outp = ctx.enter_context(tc.tile_pool(name="outp", bufs=3))

        # Per-k-tile buffers so matmuls for contraction tile k can start as
        # soon as that k-tile's xt/w DMAs land (instead of serializing the
        # full 12MB preload before the first matmul).  Loads issue in k
        # order, xt/w interleaved.
        #
        # CAUTION: with independent per-k deps, Tile is free to reorder
        # matmuls *within* one PSUM accumulation group, and a reordered
        # start=True matmul zeroes earlier partials (measured rel err 1.4).
        # The add_dep_helper chain in the matmul loop below pins each
        # (m, n4) group's k-order, which makes this split safe.
        xt_k = []
        w_k = []
        for k in range(KT):
            xk = const.tile([128, BS], _MM_DTYPE, tag=f"xt{k}")
            nc.sync.dma_start(xk[:], xt[k * 128:(k + 1) * 128, :])
            xt_k.append(xk)
            wk = const.tile([128, NOUT], _MM_DTYPE, tag=f"w{k}")
            nc.sync.dma_start(wk[:], wm[k * 128:(k + 1) * 128, :])
            w_k.append(wk)
        # Bias broadcast to all 128 partitions.
        b_sb = const.tile([128, NOUT], F32)
        bvap = bv[:]
        nc.gpsimd.dma_start(
            out=b_sb[:],
            in_=bass.AP(tensor=bvap.tensor, offset=bvap.offset, ap=[[0, 128]] + list(bvap.ap)),
        )

        for m in range(MT):
            ps = psum.tile([128, NOUT], F32)
            prev_mm = [None] * NT
            for k in range(KT):
                lhs = xt_k[k][:, m * 128:(m + 1) * 128]
                for n4 in range(NT):
                    rhs = w_k[k][:, n4 * 512:(n4 + 1) * 512]
                    mm = nc.tensor.matmul(
                        ps[:, n4 * 512:(n4 + 1) * 512],
                        lhs,
                        rhs,
                        start=(k == 0),
                        stop=(k == KT - 1),
                    )
                    if prev_mm[n4] is not None:
                        # Pin in-group accumulation order (PE executes in
                        # issue order, so a scheduling-only dep suffices).
                        tile.add_dep_helper(
                            mm.ins,
                            prev_mm[n4].ins,
                            sync=False,
                            reason="psum accumulation k-order",
                        )
                    prev_mm[n4] = mm
            o = outp.tile([128, NOUT], F32)
            nc.vector.tensor_add(o[:], ps[:], b_sb[:])
            nc.sync.dma_start(out[m * 128:(m + 1) * 128, :], o[:])
    return nc


_PROGRAM_CACHE = {}


def _get_program():
    key = str(_MM_DTYPE)
    if key not in _PROGRAM_CACHE:
        _PROGRAM_CACHE[key] = _build_program()
    return _PROGRAM_CACHE[key]


# ---------------------------------------------------------------------------
# Entry point
# ---------------------------------------------------------------------------


def kernel(x, angles_enc, angles_dec, hidden_weight, hidden_state):
    x = np.asarray(x, dtype=np.float32)
    W, bias = _host_params(
        np.asarray(angles_enc, np.float32),
        np.asarray(angles_dec, np.float32),
        np.asarray(hidden_weight, np.float32),
        np.asarray(hidden_state, np.float32),
    )

    xT = np.ascontiguousarray(x.T).astype(_IO_NP_DTYPE)  # [NF, B_FULL]
    Wd = W.astype(_IO_NP_DTYPE)
    in_maps = []
    for c in range(N_CORES):
        in_maps.append(
            {
                "xt": np.ascontiguousarray(xT[:, c * BS:(c + 1) * BS]),
                "wm": Wd,
                "bv": bias,
            }
        )

    nc = _get_program()
    res = run_bass_kernel_spmd(nc, in_maps, list(range(N_CORES)))

    out_full = np.empty((B_FULL, NOUT), np.float32)
    for c in range(N_CORES):
        out_full[c * BS:(c + 1) * BS, :] = res.results[c]["out"]

    bottleneck = np.ascontiguousarray(out_full[:, :NF])
    out = np.ascontiguousarray(out_full[:, NF:])
    return bottleneck, out
